# revision 23
# baseline (speedup 1.0000x reference)
"""Trainium2 Bass kernel for nn_CGNN_88038239634099 (GNN message passing).

Math: the edge gather/scatter-add over a fixed 64-node graph is a dense
64x64 adjacency matmul (A[dst,src] += w).  Per layer:
    h <- relu(h + A @ (h @ W_l + b_l))
Everything becomes dense matmuls over B=4096 independent samples.

v2 design (fp16 matmuls, engine-balanced):
  * All matmul operands fp16 (1 cycle/row on PE vs fp32's 4).  PSUM fp32.
  * Encoder folded into layer 0:
      h0@W0 + b0 = x (x) u + 1 (x) v   with u = W0^T w_enc, v = W0^T b_enc + b0
    -> layer-0 mm1 is a K=2 matmul from packed x; the residual h0 is
    accumulated straight into the A-mix PSUM bank by a second K=2 matmul
    with lhsT = [w_enc; b_enc].
  * Residual for layers 1,2 via identity-matmul PSUM accumulation (PE),
    so per 512-token group each engine does exactly one big op:
      PE : 4x mm1 + 4x resid + 4x A-mix   (~640ns)
      DVE: hn = p1 + b_l  (PSUM->SBUF, the forced move)  (~660ns)
      Act: h = relu(p2)   (PSUM->SBUF)                   (~610ns)
  * Software-pipelined emission (2-group lookahead; resid emitted before
    the A-mix so PE has independent work while DVE finishes hn).

Device layout: h feature-major [feat=128 part, token], token t = b*64+n.
A-mix uses MtokT = kron(I2, A.T) (2 samples per 128-token block).
Classifier contracts (n,h) via 64 accumulating matmuls, strided rhs.

Sharding: data-parallel over batch, 512 samples per core, 8 cores.
"""

import os
import sys

if "/opt/trn_rl_repo" not in sys.path:
    sys.path.insert(0, "/opt/trn_rl_repo")

import numpy as np

# experiment knobs (sim bisection only; defaults are the shipped config)
_SKIP_CLS = bool(int(os.environ.get("K_SKIP_CLS", "0")))
_LOOKAHEAD = int(os.environ.get("K_LOOKAHEAD", "2"))
_NLAYER = int(os.environ.get("K_NLAYER", "3"))
_NO_RESID = bool(int(os.environ.get("K_NO_RESID", "0")))  # timing probe only
_BPG = int(os.environ.get("K_BPG", "8"))  # 128-token blocks per group (4 or 8)

B, N, H, L, O = 4096, 64, 128, 3, 2
NCORES = 8
B_LOC = B // NCORES          # 512 samples per core
BC = B_LOC                   # one chunk
T = BC * N                   # 32768 tokens per core
NBLK = T // 128              # 256 blocks of 128 tokens
NGRP = NBLK // 4             # 64 groups of 4 blocks (512 tokens)

_CACHE = {}


def _build_module(repeat=1):
    """Build + compile the Bass/Tile module (same SPMD program on 8 cores).

    repeat>1 wraps the compute in a hardware loop that redoes the same
    work; used only for slope-based timing (outputs unchanged)."""
    import concourse.bass as bass
    import concourse.tile as tile
    from concourse import bacc, mybir

    f32 = mybir.dt.float32
    f16 = mybir.dt.float16
    AF = mybir.ActivationFunctionType
    ALU = mybir.AluOpType

    nc = bacc.Bacc(
        "TRN2",
        target_bir_lowering=False,
        debug=False,
        enable_asserts=False,
        num_devices=NCORES,
    )

    # x2: row 0 = x tokens (t = b*64+n), row 1 = ones (bias lane for K=2 mms)
    x2_d = nc.dram_tensor("x2", [2, T], f16, kind="ExternalInput").ap()
    mtokT_d = nc.dram_tensor("mtokT", [128, 128], f16, kind="ExternalInput").ap()
    i128_d = nc.dram_tensor("i128", [128, 128], f16, kind="ExternalInput").ap()
    wl_d = nc.dram_tensor("wl", [2, 128, 128], f16, kind="ExternalInput").ap()
    blrep_d = nc.dram_tensor("blrep", [2, 128, 1024], f32, kind="ExternalInput").ap()
    uv_d = nc.dram_tensor("uv", [2, 128], f16, kind="ExternalInput").ap()
    encw_d = nc.dram_tensor("encw", [2, 128], f16, kind="ExternalInput").ap()
    wc1_d = nc.dram_tensor("wc1p", [128, N * 128], f16, kind="ExternalInput").ap()
    bc1_d = nc.dram_tensor("bc1", [128, 1], f32, kind="ExternalInput").ap()
    wc2_d = nc.dram_tensor("wc2", [128, O], f16, kind="ExternalInput").ap()
    bc2_d = nc.dram_tensor("bc2", [O, 1], f32, kind="ExternalInput").ap()
    out_d = nc.dram_tensor("out_loc", [BC, O], f32, kind="ExternalOutput").ap()

    BPG = _BPG                  # 128-token blocks per group
    GW = BPG * 128              # group width in tokens
    NGRPW = NBLK // BPG         # groups per layer
    PS_BUFS = 2 if BPG == 8 else 3

    with tile.TileContext(nc) as tc:
        with (
            tc.tile_pool(name="consts", bufs=1) as cpool,
            tc.tile_pool(name="h", bufs=1) as hpool,
            tc.tile_pool(name="hn", bufs=3) as hn_pool,
            tc.tile_pool(name="hid", bufs=1) as hid_pool,
            tc.tile_pool(name="ps1", bufs=PS_BUFS, space=bass.MemorySpace.PSUM) as ps1_pool,
            tc.tile_pool(name="ps2", bufs=PS_BUFS, space=bass.MemorySpace.PSUM) as ps2_pool,
        ):
            # ---- load constants into SBUF ----
            c_x2 = cpool.tile([2, T], f16, tag="x2")
            nc.sync.dma_start(c_x2[:], x2_d[:])
            c_mtokT = cpool.tile([128, 128], f16, tag="mtokT")
            nc.sync.dma_start(c_mtokT[:], mtokT_d[:])
            c_i128 = cpool.tile([128, 128], f16, tag="i128")
            nc.sync.dma_start(c_i128[:], i128_d[:])
            c_uv = cpool.tile([2, 128], f16, tag="uv")
            nc.sync.dma_start(c_uv[:], uv_d[:])
            c_encw = cpool.tile([2, 128], f16, tag="encw")
            nc.sync.dma_start(c_encw[:], encw_d[:])
            c_wl = []
            c_bl = []
            for l in range(2):
                wt = cpool.tile([128, 128], f16, tag=f"wl{l}")
                nc.sync.dma_start(wt[:], wl_d[l])
                c_wl.append(wt)
                bt = cpool.tile([128, 1024], f32, tag=f"bl{l}")
                nc.sync.dma_start(bt[:], blrep_d[l])
                c_bl.append(bt)
            c_wc1 = cpool.tile([128, N * 128], f16, tag="wc1")
            nc.sync.dma_start(c_wc1[:], wc1_d[:])
            c_bc1 = cpool.tile([128, 1], f32, tag="bc1")
            nc.sync.dma_start(c_bc1[:], bc1_d[:])
            c_wc2 = cpool.tile([128, O], f16, tag="wc2")
            nc.sync.dma_start(c_wc2[:], wc2_d[:])
            c_bc2 = cpool.tile([O, 1], f32, tag="bc2")
            nc.sync.dma_start(c_bc2[:], bc2_d[:])

            h = hpool.tile([128, T], f16, tag="h")

            def compute():
                # ---- 3 GNN layers, software-pipelined over groups ----
                # per layer, in-flight state: p1[g] (mm1 out), p2[g]
                p1s = {}
                p2s = {}

                def mm1(l, g):
                    """W-mix (layer 0: from packed x via uv).  p1[tok, feat]."""
                    p1 = ps1_pool.tile([128, GW], f32, tag="p1")
                    for j in range(BPG):
                        dst = p1[:, j * 128 : (j + 1) * 128]
                        if l == 0:
                            nc.tensor.matmul(
                                dst,
                                c_x2[:, g * GW + j * 128 : g * GW + (j + 1) * 128],
                                c_uv[:],
                                start=True,
                                stop=True,
                            )
                        else:
                            nc.tensor.matmul(
                                dst,
                                h[:, g * GW + j * 128 : g * GW + (j + 1) * 128],
                                c_wl[l - 1][:],
                                start=True,
                                stop=True,
                            )
                    p1s[g] = p1

                def bias(l, g):
                    """DVE: hn = p1 + b_l (layer 0: plain copy; bias is in uv)."""
                    hn = hn_pool.tile([128, GW], f16, tag="hn")
                    if l == 0:
                        nc.vector.tensor_copy(hn[:], p1s[g][:])
                    else:
                        nc.vector.tensor_tensor(
                            hn[:], p1s[g][:], c_bl[l - 1][:, 0:GW], ALU.add
                        )
                    return hn

                def resid_amix(l, g, hn):
                    """p2 = residual + A-mix.  Pairwise per 128-col region:
                    CoreSim allows one pending accumulation group per bank."""
                    p2 = ps2_pool.tile([128, GW], f32, tag="p2")
                    for j in range(BPG):
                        dst = p2[:, j * 128 : (j + 1) * 128]
                        xsl = slice(g * GW + j * 128, g * GW + (j + 1) * 128)
                        if not _NO_RESID:
                            if l == 0:
                                # h0 = w_enc (x) x + b_enc (x) 1
                                nc.tensor.matmul(
                                    dst, c_encw[:], c_x2[:, xsl],
                                    start=True, stop=False,
                                )
                            else:
                                nc.tensor.matmul(
                                    dst, c_i128[:], h[:, xsl],
                                    start=True, stop=False,
                                )
                        nc.tensor.matmul(
                            dst,
                            hn[:, j * 128 : (j + 1) * 128],
                            c_mtokT[:],
                            start=_NO_RESID,
                            stop=True,
                        )
                    p2s[g] = p2

                def relu(g):
                    nc.scalar.activation(
                        h[:, g * GW : (g + 1) * GW], p2s[g][:], AF.Relu
                    )
                    del p1s[g], p2s[g]

                LA = _LOOKAHEAD
                for l in range(_NLAYER):
                    for g0 in range(min(LA, NGRPW)):
                        mm1(l, g0)
                    for g in range(NGRPW):
                        hn = bias(l, g)
                        if g + LA < NGRPW:
                            mm1(l, g + LA)
                        resid_amix(l, g, hn)
                        relu(g)

                if _SKIP_CLS:
                    lg0 = hid_pool.tile([O, BC], f32, tag="lg")
                    nc.vector.tensor_copy(lg0[:], h[0:O, 0:BC])
                    nc.sync.dma_start(out_d.rearrange("b o -> o b"), lg0[:])
                    return

                # ---- classifier: hidden = relu(h_flat @ Wc1 + bc1) ----
                # psum comes from the layer pools (sliced wide tiles)
                h3 = h[:].rearrange("p (b n) -> p n b", n=N)  # [128, N, BC]
                pcw = ps1_pool.tile([128, GW], f32, tag="p1")
                pc = pcw[:, 0:BC]
                for n in range(N):
                    nc.tensor.matmul(
                        pc,
                        c_wc1[:, n * 128 : (n + 1) * 128],
                        h3[:, n, :],
                        start=(n == 0),
                        stop=(n == N - 1),
                    )
                hid = hid_pool.tile([128, BC], f16, tag="hid")
                nc.scalar.activation(hid[:], pc, AF.Relu, bias=c_bc1[:])

                # ---- logits = hidden @ Wc2 + bc2 ----
                pow_ = ps2_pool.tile([128, GW], f32, tag="p2")
                po = pow_[0:O, 0:BC]
                nc.tensor.matmul(po, c_wc2[:], hid[:], start=True, stop=True)
                lg = hid_pool.tile([O, BC], f32, tag="lg")
                nc.scalar.activation(lg[:], po, AF.Identity, bias=c_bc2[:])
                nc.sync.dma_start(out_d.rearrange("b o -> o b"), lg[:])

            if repeat == 1:
                compute()
            else:
                with tc.For_i(0, repeat, 1):
                    compute()

    nc.compile()
    return nc


def _precompute_consts(edge_index, edge_attr, w_enc, b_enc, W_layers, b_layers,
                       Wc1, bc1, Wc2, bc2):
    src = np.asarray(edge_index[0], dtype=np.int64)
    dst = np.asarray(edge_index[1], dtype=np.int64)
    w = np.asarray(edge_attr, dtype=np.float32)[:, 0]
    A = np.zeros((N, N), dtype=np.float32)
    np.add.at(A, (dst, src), w)
    mtokT = np.kron(np.eye(2, dtype=np.float32), A.T)
    W = np.asarray(W_layers, dtype=np.float32)
    bl = np.asarray(b_layers, dtype=np.float32)
    we = np.asarray(w_enc, np.float32)
    be = np.asarray(b_enc, np.float32)
    u = W[0].T @ we
    v = W[0].T @ be + bl[0]
    wc1 = np.asarray(Wc1, dtype=np.float32)
    # lhsT block n at columns n*128: wc1p[p, n*128+k] = Wc1[n*128+p, k]
    wc1p = wc1.reshape(N, 128, 128).transpose(1, 0, 2).reshape(128, N * 128)
    consts = {
        "mtokT": np.ascontiguousarray(mtokT.astype(np.float16)),
        "i128": np.eye(128, dtype=np.float16),
        "uv": np.ascontiguousarray(np.stack([u, v], 0).astype(np.float16)),
        "encw": np.ascontiguousarray(np.stack([we, be], 0).astype(np.float16)),
        "wl": np.ascontiguousarray(W[1:3].astype(np.float16)),
        "blrep": np.ascontiguousarray(
            np.broadcast_to(np.tile(bl[1:3], (1, 8))[:, None, :], (2, 128, 1024))
        ).astype(np.float32),
        "wc1p": np.ascontiguousarray(wc1p.astype(np.float16)),
        "bc1": np.ascontiguousarray(np.asarray(bc1, np.float32).reshape(128, 1)),
        "wc2": np.ascontiguousarray(np.asarray(Wc2, np.float32).astype(np.float16)),
        "bc2": np.ascontiguousarray(np.asarray(bc2, np.float32).reshape(O, 1)),
    }
    return consts


def _get_nc(repeat=1):
    key = ("nc", repeat)
    if key not in _CACHE:
        _CACHE[key] = _build_module(repeat)
    return _CACHE[key]


def _make_in_maps(inputs):
    consts = _precompute_consts(
        inputs["edge_index"], inputs["edge_attr"], inputs["w_enc"],
        inputs["b_enc"], inputs["W_layers"], inputs["b_layers"],
        inputs["Wc1"], inputs["bc1"], inputs["Wc2"], inputs["bc2"],
    )
    x = np.asarray(inputs["x"], dtype=np.float32)
    in_maps = []
    for core in range(NCORES):
        flat = x[core * B_LOC : (core + 1) * B_LOC].reshape(T)  # t = b*64+n
        x2 = np.empty((2, T), dtype=np.float16)
        x2[0] = flat.astype(np.float16)
        x2[1] = np.float16(1.0)
        m = {"x2": np.ascontiguousarray(x2)}
        m.update(consts)
        in_maps.append(m)
    return in_maps


def _run(inputs, trace=False):
    """inputs: full unsharded dict. Returns (logits [B,O], BassKernelResults)."""
    from concourse import bass_utils

    nc = _get_nc()
    in_maps = _make_in_maps(inputs)
    res = bass_utils.run_bass_kernel_spmd(
        nc, in_maps, core_ids=list(range(NCORES)), trace=trace
    )
    out = np.concatenate(
        [res.results[c]["out_loc"].reshape(B_LOC, O) for c in range(NCORES)], axis=0
    )
    return out, res


def kernel(**inputs):
    out, _ = _run(inputs, trace=False)
    return out


# revision 25
# speedup vs baseline: 1.3444x; 1.3444x over previous
"""Trainium2 Bass kernel for nn_CGNN_88038239634099 (GNN message passing).

Math: the edge gather/scatter-add over a fixed 64-node graph is a dense
64x64 adjacency matmul (A[dst,src] += w).  Per layer:
    h <- relu(h + A @ (h @ W_l + b_l))
Everything becomes dense matmuls over B=4096 independent samples.

v2 design (fp16 matmuls, engine-balanced):
  * All matmul operands fp16 (1 cycle/row on PE vs fp32's 4).  PSUM fp32.
  * Encoder folded into layer 0:
      h0@W0 + b0 = x (x) u + 1 (x) v   with u = W0^T w_enc, v = W0^T b_enc + b0
    -> layer-0 mm1 is a K=2 matmul from packed x; the residual h0 is
    accumulated straight into the A-mix PSUM bank by a second K=2 matmul
    with lhsT = [w_enc; b_enc].
  * Residual for layers 1,2 via identity-matmul PSUM accumulation (PE),
    so per 1024-token group (2-bank PSUM tiles) each engine does one big op:
      PE : 8x mm1 + 8x (resid; A-mix) pairs   (~1280ns)
      DVE: hn = p1 + b_l  (PSUM->SBUF, the forced move)  (~1190ns)
      Act: h = relu(p2)   (PSUM->SBUF)                   (~1040ns)
    resid/A-mix are emitted pairwise per 128-col region because only one
    PSUM accumulation group may be pending per bank.
  * Software-pipelined emission: bias(g) -> mm1(g+2) -> pairs(g) -> relu(g)
    keeps the mm1 feeding bias(g+2) off the pairs' critical path.

Device layout: h feature-major [feat=128 part, token], token t = b*64+n.
A-mix uses MtokT = kron(I2, A.T) (2 samples per 128-token block).
Classifier contracts (n,h) via 64 accumulating matmuls, strided rhs.

Sharding: data-parallel over batch, 512 samples per core, 8 cores.
"""

import os
import sys

if "/opt/trn_rl_repo" not in sys.path:
    sys.path.insert(0, "/opt/trn_rl_repo")

import numpy as np

# experiment knobs (sim bisection only; defaults are the shipped config)
_SKIP_CLS = bool(int(os.environ.get("K_SKIP_CLS", "0")))
_LOOKAHEAD = int(os.environ.get("K_LOOKAHEAD", "2"))
_NLAYER = int(os.environ.get("K_NLAYER", "3"))
_NO_RESID = bool(int(os.environ.get("K_NO_RESID", "0")))  # timing probe only
_BPG = int(os.environ.get("K_BPG", "8"))  # 128-token blocks per group (4 or 8)

B, N, H, L, O = 4096, 64, 128, 3, 2
NCORES = 8
B_LOC = B // NCORES          # 512 samples per core
BC = B_LOC                   # one chunk
T = BC * N                   # 32768 tokens per core
NBLK = T // 128              # 256 blocks of 128 tokens
NGRP = NBLK // 4             # 64 groups of 4 blocks (512 tokens)

_CACHE = {}


def _build_module(repeat=1):
    """Build + compile the Bass/Tile module (same SPMD program on 8 cores).

    repeat>1 wraps the compute in a hardware loop that redoes the same
    work; used only for slope-based timing (outputs unchanged)."""
    import concourse.bass as bass
    import concourse.tile as tile
    from concourse import bacc, mybir

    f32 = mybir.dt.float32
    f16 = mybir.dt.float16
    AF = mybir.ActivationFunctionType
    ALU = mybir.AluOpType

    nc = bacc.Bacc(
        "TRN2",
        target_bir_lowering=False,
        debug=False,
        enable_asserts=False,
        num_devices=NCORES,
    )

    # x2: row 0 = x tokens (t = b*64+n), row 1 = ones (bias lane for K=2 mms)
    x2_d = nc.dram_tensor("x2", [2, T], f16, kind="ExternalInput").ap()
    mtokT_d = nc.dram_tensor("mtokT", [128, 128], f16, kind="ExternalInput").ap()
    i128_d = nc.dram_tensor("i128", [128, 128], f16, kind="ExternalInput").ap()
    wl_d = nc.dram_tensor("wl", [2, 128, 128], f16, kind="ExternalInput").ap()
    blrep_d = nc.dram_tensor("blrep", [2, 128, 1024], f32, kind="ExternalInput").ap()
    uv_d = nc.dram_tensor("uv", [2, 128], f16, kind="ExternalInput").ap()
    encw_d = nc.dram_tensor("encw", [2, 128], f16, kind="ExternalInput").ap()
    wc1_d = nc.dram_tensor("wc1p", [128, N * 128], f16, kind="ExternalInput").ap()
    bc1_d = nc.dram_tensor("bc1", [128, 1], f32, kind="ExternalInput").ap()
    wc2_d = nc.dram_tensor("wc2", [128, O], f16, kind="ExternalInput").ap()
    bc2_d = nc.dram_tensor("bc2", [O, 1], f32, kind="ExternalInput").ap()
    out_d = nc.dram_tensor("out_loc", [BC, O], f32, kind="ExternalOutput").ap()

    BPG = _BPG                  # 128-token blocks per group
    GW = BPG * 128              # group width in tokens
    NGRPW = NBLK // BPG         # groups per layer
    PS_BUFS = 2 if BPG == 8 else 3

    with tile.TileContext(nc) as tc:
        with (
            tc.tile_pool(name="consts", bufs=1) as cpool,
            tc.tile_pool(name="h", bufs=1) as hpool,
            tc.tile_pool(name="hn", bufs=3) as hn_pool,
            tc.tile_pool(name="hid", bufs=1) as hid_pool,
            tc.tile_pool(name="ps1", bufs=PS_BUFS, space=bass.MemorySpace.PSUM) as ps1_pool,
            tc.tile_pool(name="ps2", bufs=PS_BUFS, space=bass.MemorySpace.PSUM) as ps2_pool,
        ):
            # ---- load constants into SBUF ----
            c_x2 = cpool.tile([2, T], f16, tag="x2")
            nc.sync.dma_start(c_x2[:], x2_d[:])
            c_mtokT = cpool.tile([128, 128], f16, tag="mtokT")
            nc.sync.dma_start(c_mtokT[:], mtokT_d[:])
            c_i128 = cpool.tile([128, 128], f16, tag="i128")
            nc.sync.dma_start(c_i128[:], i128_d[:])
            c_uv = cpool.tile([2, 128], f16, tag="uv")
            nc.sync.dma_start(c_uv[:], uv_d[:])
            c_encw = cpool.tile([2, 128], f16, tag="encw")
            nc.sync.dma_start(c_encw[:], encw_d[:])
            c_wl = []
            c_bl = []
            for l in range(2):
                wt = cpool.tile([128, 128], f16, tag=f"wl{l}")
                nc.sync.dma_start(wt[:], wl_d[l])
                c_wl.append(wt)
                bt = cpool.tile([128, 1024], f32, tag=f"bl{l}")
                nc.sync.dma_start(bt[:], blrep_d[l])
                c_bl.append(bt)
            c_wc1 = cpool.tile([128, N * 128], f16, tag="wc1")
            nc.sync.dma_start(c_wc1[:], wc1_d[:])
            c_bc1 = cpool.tile([128, 1], f32, tag="bc1")
            nc.sync.dma_start(c_bc1[:], bc1_d[:])
            c_wc2 = cpool.tile([128, O], f16, tag="wc2")
            nc.sync.dma_start(c_wc2[:], wc2_d[:])
            c_bc2 = cpool.tile([O, 1], f32, tag="bc2")
            nc.sync.dma_start(c_bc2[:], bc2_d[:])

            h = hpool.tile([128, T], f16, tag="h")

            def compute():
                # ---- 3 GNN layers, software-pipelined over groups ----
                # per layer, in-flight state: p1[g] (mm1 out), p2[g]
                p1s = {}
                p2s = {}

                def mm1(l, g):
                    """W-mix (layer 0: from packed x via uv).  p1[tok, feat]."""
                    p1 = ps1_pool.tile([128, GW], f32, tag="p1")
                    for j in range(BPG):
                        dst = p1[:, j * 128 : (j + 1) * 128]
                        if l == 0:
                            nc.tensor.matmul(
                                dst,
                                c_x2[:, g * GW + j * 128 : g * GW + (j + 1) * 128],
                                c_uv[:],
                                start=True,
                                stop=True,
                            )
                        else:
                            nc.tensor.matmul(
                                dst,
                                h[:, g * GW + j * 128 : g * GW + (j + 1) * 128],
                                c_wl[l - 1][:],
                                start=True,
                                stop=True,
                            )
                    p1s[g] = p1

                def bias(l, g):
                    """DVE: hn = p1 + b_l (layer 0: plain copy; bias is in uv)."""
                    hn = hn_pool.tile([128, GW], f16, tag="hn")
                    if l == 0:
                        nc.vector.tensor_copy(hn[:], p1s[g][:])
                    else:
                        nc.vector.tensor_tensor(
                            hn[:], p1s[g][:], c_bl[l - 1][:, 0:GW], ALU.add
                        )
                    return hn

                def resid_amix(l, g, hn):
                    """p2 = residual + A-mix.  Pairwise per 128-col region:
                    CoreSim allows one pending accumulation group per bank."""
                    p2 = ps2_pool.tile([128, GW], f32, tag="p2")
                    for j in range(BPG):
                        dst = p2[:, j * 128 : (j + 1) * 128]
                        xsl = slice(g * GW + j * 128, g * GW + (j + 1) * 128)
                        if not _NO_RESID:
                            if l == 0:
                                # h0 = w_enc (x) x + b_enc (x) 1
                                nc.tensor.matmul(
                                    dst, c_encw[:], c_x2[:, xsl],
                                    start=True, stop=False,
                                )
                            else:
                                nc.tensor.matmul(
                                    dst, c_i128[:], h[:, xsl],
                                    start=True, stop=False,
                                )
                        nc.tensor.matmul(
                            dst,
                            hn[:, j * 128 : (j + 1) * 128],
                            c_mtokT[:],
                            start=_NO_RESID,
                            stop=True,
                        )
                    p2s[g] = p2

                def relu(g):
                    nc.scalar.activation(
                        h[:, g * GW : (g + 1) * GW], p2s[g][:], AF.Relu
                    )
                    del p1s[g], p2s[g]

                LA = _LOOKAHEAD
                for l in range(_NLAYER):
                    for g0 in range(min(LA, NGRPW)):
                        mm1(l, g0)
                    mm1_first = bool(int(os.environ.get("K_MM1_FIRST", "1")))
                    for g in range(NGRPW):
                        hn = bias(l, g)
                        if mm1_first:
                            if g + LA < NGRPW:
                                mm1(l, g + LA)
                            resid_amix(l, g, hn)
                        else:
                            resid_amix(l, g, hn)
                            if g + LA < NGRPW:
                                mm1(l, g + LA)
                        relu(g)

                if _SKIP_CLS:
                    lg0 = hid_pool.tile([O, BC], f32, tag="lg")
                    nc.vector.tensor_copy(lg0[:], h[0:O, 0:BC])
                    nc.sync.dma_start(out_d.rearrange("b o -> o b"), lg0[:])
                    return

                # ---- classifier: hidden = relu(h_flat @ Wc1 + bc1) ----
                # psum comes from the layer pools (sliced wide tiles)
                h3 = h[:].rearrange("p (b n) -> p n b", n=N)  # [128, N, BC]
                pcw = ps1_pool.tile([128, GW], f32, tag="p1")
                pc = pcw[:, 0:BC]
                for n in range(N):
                    nc.tensor.matmul(
                        pc,
                        c_wc1[:, n * 128 : (n + 1) * 128],
                        h3[:, n, :],
                        start=(n == 0),
                        stop=(n == N - 1),
                    )
                hid = hid_pool.tile([128, BC], f16, tag="hid")
                nc.scalar.activation(hid[:], pc, AF.Relu, bias=c_bc1[:])

                # ---- logits = hidden @ Wc2 + bc2 ----
                pow_ = ps2_pool.tile([128, GW], f32, tag="p2")
                po = pow_[0:O, 0:BC]
                nc.tensor.matmul(po, c_wc2[:], hid[:], start=True, stop=True)
                lg = hid_pool.tile([O, BC], f32, tag="lg")
                nc.scalar.activation(lg[:], po, AF.Identity, bias=c_bc2[:])
                nc.sync.dma_start(out_d.rearrange("b o -> o b"), lg[:])

            if repeat == 1:
                compute()
            else:
                with tc.For_i(0, repeat, 1):
                    compute()

    nc.compile()
    return nc


def _precompute_consts(edge_index, edge_attr, w_enc, b_enc, W_layers, b_layers,
                       Wc1, bc1, Wc2, bc2):
    src = np.asarray(edge_index[0], dtype=np.int64)
    dst = np.asarray(edge_index[1], dtype=np.int64)
    w = np.asarray(edge_attr, dtype=np.float32)[:, 0]
    A = np.zeros((N, N), dtype=np.float32)
    np.add.at(A, (dst, src), w)
    mtokT = np.kron(np.eye(2, dtype=np.float32), A.T)
    W = np.asarray(W_layers, dtype=np.float32)
    bl = np.asarray(b_layers, dtype=np.float32)
    we = np.asarray(w_enc, np.float32)
    be = np.asarray(b_enc, np.float32)
    u = W[0].T @ we
    v = W[0].T @ be + bl[0]
    wc1 = np.asarray(Wc1, dtype=np.float32)
    # lhsT block n at columns n*128: wc1p[p, n*128+k] = Wc1[n*128+p, k]
    wc1p = wc1.reshape(N, 128, 128).transpose(1, 0, 2).reshape(128, N * 128)
    consts = {
        "mtokT": np.ascontiguousarray(mtokT.astype(np.float16)),
        "i128": np.eye(128, dtype=np.float16),
        "uv": np.ascontiguousarray(np.stack([u, v], 0).astype(np.float16)),
        "encw": np.ascontiguousarray(np.stack([we, be], 0).astype(np.float16)),
        "wl": np.ascontiguousarray(W[1:3].astype(np.float16)),
        "blrep": np.ascontiguousarray(
            np.broadcast_to(np.tile(bl[1:3], (1, 8))[:, None, :], (2, 128, 1024))
        ).astype(np.float32),
        "wc1p": np.ascontiguousarray(wc1p.astype(np.float16)),
        "bc1": np.ascontiguousarray(np.asarray(bc1, np.float32).reshape(128, 1)),
        "wc2": np.ascontiguousarray(np.asarray(Wc2, np.float32).astype(np.float16)),
        "bc2": np.ascontiguousarray(np.asarray(bc2, np.float32).reshape(O, 1)),
    }
    return consts


def _get_nc(repeat=1):
    key = ("nc", repeat)
    if key not in _CACHE:
        _CACHE[key] = _build_module(repeat)
    return _CACHE[key]


def _make_in_maps(inputs):
    consts = _precompute_consts(
        inputs["edge_index"], inputs["edge_attr"], inputs["w_enc"],
        inputs["b_enc"], inputs["W_layers"], inputs["b_layers"],
        inputs["Wc1"], inputs["bc1"], inputs["Wc2"], inputs["bc2"],
    )
    x = np.asarray(inputs["x"], dtype=np.float32)
    in_maps = []
    for core in range(NCORES):
        flat = x[core * B_LOC : (core + 1) * B_LOC].reshape(T)  # t = b*64+n
        x2 = np.empty((2, T), dtype=np.float16)
        x2[0] = flat.astype(np.float16)
        x2[1] = np.float16(1.0)
        m = {"x2": np.ascontiguousarray(x2)}
        m.update(consts)
        in_maps.append(m)
    return in_maps


def _run(inputs, trace=False):
    """inputs: full unsharded dict. Returns (logits [B,O], BassKernelResults)."""
    from concourse import bass_utils

    nc = _get_nc()
    in_maps = _make_in_maps(inputs)
    res = bass_utils.run_bass_kernel_spmd(
        nc, in_maps, core_ids=list(range(NCORES)), trace=trace
    )
    out = np.concatenate(
        [res.results[c]["out_loc"].reshape(B_LOC, O) for c in range(NCORES)], axis=0
    )
    return out, res


def kernel(**inputs):
    out, _ = _run(inputs, trace=False)
    return out
